# revision 1
# baseline (speedup 1.0000x reference)
"""NetVLAD Trainium2 kernel (8 NeuronCores, data-parallel over points).

Strategy:
  - Host: stable-sort points by batch_id, pad each batch to 8*128*T_b rows
    (pad rows = e0 unit vectors, corrected exactly after the all-reduce),
    distribute T_b tiles per batch to every core -> identical SPMD program.
    Feat is shipped twice in bf16: natural [Np,256] (aggregation operand)
    and transposed [256,Np] (logits matmul operand) - same total bytes as
    one fp32 copy.
  - Device per 128-point tile: logits = featT.T @ Wt (bf16 matmul, fp32
    psum); sumsq via fused tensor_tensor_reduce; r=1/||feat|| via Ln/Exp
    (same ACT table set as the softmax Exp); t3 = psum*r + bias in one
    scalar_tensor_tensor; row max; Exp with per-partition bias and fused
    Z accumulation; soft2 = e * (r/Z); aggregation matmul soft2.T @
    [feat, ||feat||] accumulated per-batch in PSUM half-banks.
  - AllReduce the [B,K,257] partials, subtract pad correction, VLAD
    (S*c - A), intra-normalize, FC with per-core output slice (col-packed
    matmuls), AllGather, final l2norm.
"""

import numpy as np
import ml_dtypes

BF16 = ml_dtypes.bfloat16

N, C, K, B, OUT = 200000, 256, 64, 8, 1024
NCORES = 8
P = 128
G = 8  # tiles per group for batched small ops

_compiled_cache = {}
PROFILE = False       # set True to capture an NTFF profile (test harness only)
LAST_RESULT = None    # BassKernelResults of the most recent run


# ----------------------------------------------------------------------------
# Host-side planning
# ----------------------------------------------------------------------------

def _plan(feat, batch_ids):
    """Sort by batch, pad each batch to NCORES*P*T_b rows, build per-core
    shards in both layouts plus pad bookkeeping."""
    order = np.argsort(batch_ids, kind="stable")
    feat_s = feat[order]
    bids_s = batch_ids[order]
    counts = np.bincount(batch_ids, minlength=B)

    Ts = [int(np.ceil(c / (NCORES * P))) for c in counts]
    n_pad = [NCORES * P * Ts[b] - int(counts[b]) for b in range(B)]

    pad_row = np.zeros((C,), np.float32)
    pad_row[0] = 1.0

    # global per-batch arrays, padded
    per_core = [[] for _ in range(NCORES)]
    off = 0
    for b in range(B):
        nb = int(counts[b])
        fb = feat_s[off:off + nb]
        off += nb
        tot = NCORES * P * Ts[b]
        if n_pad[b]:
            fb = np.concatenate([fb, np.broadcast_to(pad_row, (n_pad[b], C))], 0)
        fb = fb.reshape(NCORES, P * Ts[b], C)
        for i in range(NCORES):
            per_core[i].append(fb[i])

    core_feat = [np.concatenate(chunks, 0) for chunks in per_core]  # [NP, C] f32
    return core_feat, Ts, n_pad


def _pad_correction(conv_w, conv_b, n_pad):
    """Exact contribution of one e0 pad row through the device pipeline."""
    w_bf = conv_w.astype(BF16)
    raw = w_bf[:, 0].astype(np.float32)          # feat=e0 -> raw logits = W[:,0]
    t3 = raw * 1.0 + conv_b.astype(np.float32)   # r = 1 (sumsq = 1)
    m = t3.max()
    e = np.exp(t3 - m).astype(BF16)
    Z = e.astype(np.float32).sum()
    soft2 = (e.astype(np.float32) * (np.float32(1.0) / Z)).astype(BF16)
    s2f = soft2.astype(np.float32)
    corr = np.zeros((B * K, C + 1), np.float32)
    for b in range(B):
        corr[b * K:(b + 1) * K, 0] = n_pad[b] * s2f   # feat_ext col 0 (=1)
        corr[b * K:(b + 1) * K, C] = n_pad[b] * s2f   # norm col (=1)
    return corr


# ----------------------------------------------------------------------------
# Device program
# ----------------------------------------------------------------------------

def _build_nc(Ts):
    import concourse.bass as bass
    import concourse.bacc as bacc
    import concourse.mybir as mybir
    from concourse import tile

    dt = mybir.dt
    AF = mybir.ActivationFunctionType
    ALU = mybir.AluOpType

    TT = sum(Ts)            # tiles per core
    NP = TT * P             # points per core
    tile_batch = []         # batch id of each tile
    for b in range(B):
        tile_batch += [b] * Ts[b]

    nc = bacc.Bacc(
        "TRN2", target_bir_lowering=False, debug=False, num_devices=NCORES
    )

    # --- I/O ---
    featN_d = nc.dram_tensor("featN", [P, TT, C], dt.bfloat16, kind="ExternalInput").ap()
    featT_d = nc.dram_tensor("featT", [C, NP], dt.bfloat16, kind="ExternalInput").ap()
    wt_d = nc.dram_tensor("wt", [C, K], dt.bfloat16, kind="ExternalInput").ap()
    bb_d = nc.dram_tensor("bb", [P, K], dt.float32, kind="ExternalInput").ap()
    cent2_d = nc.dram_tensor("cent2", [P, C], dt.bfloat16, kind="ExternalInput").ap()
    corr_d = nc.dram_tensor("corr", [B * K, C + 1], dt.float32, kind="ExternalInput").ap()
    fwt_d = nc.dram_tensor("fwt", [P, K * C], dt.bfloat16, kind="ExternalInput").ap()
    fbb_d = nc.dram_tensor("fbb", [B, OUT // NCORES], dt.float32, kind="ExternalInput").ap()
    ident_d = nc.dram_tensor("ident", [P, P], dt.bfloat16, kind="ExternalInput").ap()
    sel_d = nc.dram_tensor("sel", [P, B], dt.float32, kind="ExternalInput").ap()
    out_d = nc.dram_tensor("out", [B, OUT], dt.float32, kind="ExternalOutput").ap()

    OSL = OUT // NCORES  # 128 output slice per core

    with tile.TileContext(nc) as tc:
        with (
            tc.tile_pool(name="const", bufs=1) as cpool,
            tc.tile_pool(name="dram", bufs=1, space="DRAM") as dram,
        ):
            wt_sb = cpool.tile([P, 2, K], dt.bfloat16, name="wt_sb")
            for h in range(2):
                nc.sync.dma_start(out=wt_sb[:, h, :],
                                  in_=wt_d[h * P:(h + 1) * P, :])
            bb_sb = cpool.tile([P, K], dt.float32, name="bb_sb")
            nc.sync.dma_start(out=bb_sb[:, :], in_=bb_d[:, :])
            cent2_sb = cpool.tile([P, C], dt.bfloat16, name="cent2_sb")
            nc.sync.dma_start(out=cent2_sb[:, :], in_=cent2_d[:, :])
            ident_sb = cpool.tile([P, P], dt.bfloat16, name="ident_sb")
            nc.sync.dma_start(out=ident_sb[:, :], in_=ident_d[:, :])
            fbb_sb = cpool.tile([B, OSL], dt.float32, name="fbb_sb")
            nc.sync.dma_start(out=fbb_sb[:, :], in_=fbb_d[:, :])
            fwt_sb = cpool.tile([P, K * C], dt.bfloat16, name="fwt_sb")
            for q in range(4):
                qs = K * C // 4
                nc.scalar.dma_start(out=fwt_sb[:, q * qs:(q + 1) * qs],
                                    in_=fwt_d[:, q * qs:(q + 1) * qs])

            # vlad-phase tiles that outlive the main-loop pools
            vpool_ctx = tc.tile_pool(name="vlad", bufs=1)
            vpool = vpool_ctx.__enter__()
            vbf = [vpool.tile([P, C], dt.bfloat16, name=f"vbf{i}")
                   for i in range(4)]
            ssv = vpool.tile([P, 4], dt.float32, name="ssv")
            lnv = vpool.tile([P, 4], dt.float32, name="lnv")
            rnv = vpool.tile([P, 4], dt.float32, name="rnv")

            # ---------------- main point loop ----------------
            with (
                tc.tile_pool(name="aggp", bufs=1, space="PSUM") as aggp,
                tc.tile_pool(name="psl", bufs=4, space="PSUM") as pslp,
                tc.tile_pool(name="grp", bufs=6) as gpool,
                tc.tile_pool(name="tl", bufs=10) as tpool,
                tc.tile_pool(name="scr", bufs=4) as spool,
            ):
                agg = [aggp.tile([P, C + 1], dt.float32, name=f"agg{i}")
                       for i in range(4)]

                t = 0
                last_tile_of_pair = [sum(Ts[:2 * pp + 2]) - 1 for pp in range(4)]
                evaced = [False] * 4

                part_d = dram.tile([B * K, C + 1], dt.float32, name="part_d")
                red_d = dram.tile([B * K, C + 1], dt.float32, name="red_d")

                while t < TT:
                    g_size = min(G, TT - t)
                    featT_g = gpool.tile([P, 2, G * P], dt.bfloat16, name="featT_g")
                    featN_g = gpool.tile([P, G, C + 1], dt.bfloat16, name="featN_g")
                    ssq_g = gpool.tile([P, G], dt.float32, name="ssq_g")
                    lns_g = gpool.tile([P, G], dt.float32, name="lns_g")
                    r_g = gpool.tile([P, G], dt.float32, name="r_g")
                    negm_g = gpool.tile([P, G], dt.float32, name="negm_g")
                    z_g = gpool.tile([P, G], dt.float32, name="z_g")
                    rz_g = gpool.tile([P, G], dt.float32, name="rz_g")
                    s2_g = gpool.tile([P, G], dt.float32, name="s2_g")
                    t3_g = gpool.tile([P, G, K], dt.float32, name="t3_g")
                    e_g = gpool.tile([P, G * K], dt.bfloat16, name="e_g")

                    # featT: two [128, g*128] contiguous slabs (c-halves)
                    for h in range(2):
                        nc.sync.dma_start(
                            out=featT_g[:, h, 0:g_size * P],
                            in_=featT_d[h * P:(h + 1) * P, t * P:(t + g_size) * P],
                        )
                    # featN: one DMA for the whole group (ACT-seq HWDGE)
                    nc.scalar.dma_start(
                        out=featN_g[:, 0:g_size, 0:C],
                        in_=featN_d[:, t:t + g_size, :],
                    )

                    # sumsq per tile (fused square+reduce), DVE/ACT alternating
                    for g in range(g_size):
                        ttr_scr = spool.tile([P, C], dt.bfloat16, name="ttr_scr")
                        if g % 2 == 0:
                            nc.vector.scalar_tensor_tensor(
                                out=ttr_scr[:, :],
                                in0=featN_g[:, g, 0:C],
                                scalar=1.0,
                                in1=featN_g[:, g, 0:C],
                                op0=ALU.mult,
                                op1=ALU.mult,
                                accum_out=ssq_g[:, g:g + 1],
                            )
                        else:
                            nc.scalar.activation(
                                ttr_scr[:, :], featN_g[:, g, 0:C], AF.Square,
                                accum_out=ssq_g[:, g:g + 1],
                            )

                    # r = exp(-0.5 ln(ssq)); norm = ssq * r  (batched)
                    nc.scalar.activation(
                        lns_g[:, 0:g_size], ssq_g[:, 0:g_size], AF.Ln)
                    nc.scalar.activation(
                        r_g[:, 0:g_size], lns_g[:, 0:g_size], AF.Exp, scale=-0.5)
                    # norm column into featN_g[:, g, C]
                    nc.vector.tensor_tensor(
                        out=featN_g[:, 0:g_size, C],
                        in0=ssq_g[:, 0:g_size],
                        in1=r_g[:, 0:g_size],
                        op=ALU.mult,
                    )

                    for g in range(g_size):
                        psumL = pslp.tile([P, K], dt.float32, name="psumL")
                        # logits matmuls: psumL = featT0.T@Wt0 + featT1.T@Wt1
                        nc.tensor.matmul(
                            psumL[:, :],
                            lhsT=featT_g[:, 0, g * P:(g + 1) * P],
                            rhs=wt_sb[:, 0, :],
                            start=True, stop=False,
                        )
                        nc.tensor.matmul(
                            psumL[:, :],
                            lhsT=featT_g[:, 1, g * P:(g + 1) * P],
                            rhs=wt_sb[:, 1, :],
                            start=False, stop=True,
                        )
                        # t3 = psumL * r + bias
                        nc.vector.scalar_tensor_tensor(
                            out=t3_g[:, g, :],
                            in0=psumL[:, :],
                            scalar=r_g[:, g:g + 1],
                            in1=bb_sb[:, :],
                            op0=ALU.mult,
                            op1=ALU.add,
                        )

                    # batched negated row max over the whole group
                    nc.vector.tensor_reduce(
                        out=negm_g[:, 0:g_size],
                        in_=t3_g[:, 0:g_size, :],
                        axis=mybir.AxisListType.X,
                        op=ALU.max,
                        negate=True,
                    )
                    for g in range(g_size):
                        # e = exp(t3 - m)
                        nc.scalar.activation(
                            e_g[:, g * K:(g + 1) * K], t3_g[:, g, :], AF.Exp,
                            bias=negm_g[:, g:g + 1],
                        )
                    # batched Z = sum_k e
                    nc.vector.tensor_reduce(
                        out=z_g[:, 0:g_size],
                        in_=e_g.rearrange("p (g k) -> p g k", k=K)[:, 0:g_size, :],
                        axis=mybir.AxisListType.X,
                        op=ALU.add,
                    )
                    # s2 = r / Z (batched)
                    nc.vector.reciprocal(rz_g[:, 0:g_size], z_g[:, 0:g_size])
                    nc.vector.tensor_tensor(
                        out=s2_g[:, 0:g_size],
                        in0=r_g[:, 0:g_size],
                        in1=rz_g[:, 0:g_size],
                        op=ALU.mult,
                    )

                    for g in range(g_size):
                        tt = t + g
                        bb_idx = tile_batch[tt]
                        # soft2 = e * s2 (2 of 8 tiles on ACT to unload DVE)
                        soft2 = tpool.tile([P, K], dt.bfloat16, name="soft2")
                        if g % 4 == 3:
                            nc.scalar.mul(
                                soft2[:, :], e_g[:, g * K:(g + 1) * K],
                                s2_g[:, g:g + 1])
                        else:
                            nc.vector.tensor_scalar(
                                out=soft2[:, :],
                                in0=e_g[:, g * K:(g + 1) * K],
                                scalar1=s2_g[:, g:g + 1],
                                scalar2=None,
                                op0=ALU.mult,
                            )
                        # aggregation matmul into per-batch half bank
                        pair, half = bb_idx // 2, bb_idx % 2
                        first = (tt == 0) or (tile_batch[tt - 1] != bb_idx)
                        last = (tt == TT - 1) or (tile_batch[tt + 1] != bb_idx)
                        nc.tensor.matmul(
                            agg[pair][half * K:(half + 1) * K, :],
                            lhsT=soft2[:, :],
                            rhs=featN_g[:, g, :],
                            start=first, stop=last,
                            tile_position=(0, half * K),
                        )
                    t += g_size

                    # early evac + all-reduce per completed batch pair
                    for pp in range(4):
                        if not evaced[pp] and last_tile_of_pair[pp] < t:
                            evaced[pp] = True
                            ev = tpool.tile([P, C + 1], dt.float32, name="ev",
                                            tag="ev", bufs=2)
                            nc.scalar.copy(ev[:, :], agg[pp][:, :])
                            nc.sync.dma_start(
                                out=part_d[pp * P:(pp + 1) * P, :], in_=ev[:, :])
                            nc.gpsimd.collective_compute(
                                "AllReduce",
                                ALU.add,
                                replica_groups=[list(range(NCORES))],
                                ins=[part_d[pp * P:(pp + 1) * P, :]],
                                outs=[red_d[pp * P:(pp + 1) * P, :]],
                            )
                assert all(evaced)

            # ---------------- fc ----------------
            with (
                tc.tile_pool(name="fin", bufs=1) as fpool,
                tc.tile_pool(name="fps", bufs=2, space="PSUM") as fpsum,
                tc.tile_pool(name="fcp", bufs=1, space="PSUM") as fcps,
            ):
                vT = [fpool.tile([P, 4 * P], dt.bfloat16, name=f"vT{h}")
                      for h in range(2)]
                for i in range(4):
                    ared = fpool.tile([P, C + 1], dt.float32, name="ared",
                                      tag="ared", bufs=2)
                    corr_sb = fpool.tile([P, C + 1], dt.float32, name="corr_sb",
                                         tag="corr_sb", bufs=2)
                    nc.sync.dma_start(out=ared[:, :],
                                      in_=red_d[i * P:(i + 1) * P, :])
                    nc.sync.dma_start(out=corr_sb[:, :],
                                      in_=corr_d[i * P:(i + 1) * P, :])
                    nc.vector.tensor_tensor(
                        out=ared[:, :], in0=ared[:, :], in1=corr_sb[:, :],
                        op=ALU.subtract)
                    nv = fpool.tile([P, C], dt.float32, name="nv", tag="nv",
                                    bufs=2)
                    nc.vector.scalar_tensor_tensor(
                        out=nv[:, :], in0=cent2_sb[:, :],
                        scalar=ared[:, C:C + 1], in1=ared[:, 0:C],
                        op0=ALU.mult, op1=ALU.subtract)
                    nvs = fpool.tile([P, C], dt.float32, name="nvs", tag="nvs",
                                     bufs=2)
                    nc.vector.scalar_tensor_tensor(
                        out=nvs[:, :], in0=nv[:, :], scalar=1.0, in1=nv[:, :],
                        op0=ALU.mult, op1=ALU.mult,
                        accum_out=ssv[:, i:i + 1])
                    nc.vector.tensor_scalar_max(
                        ssv[:, i:i + 1], ssv[:, i:i + 1], 1e-24)
                    nc.scalar.activation(lnv[:, i:i + 1], ssv[:, i:i + 1], AF.Ln)
                    nc.scalar.activation(rnv[:, i:i + 1], lnv[:, i:i + 1],
                                         AF.Exp, scale=-0.5)
                    nc.vector.tensor_scalar(
                        out=vbf[i][:, :], in0=nv[:, :],
                        scalar1=rnv[:, i:i + 1], scalar2=None, op0=ALU.mult)
                    # transpose the two c-halves into vT buffers
                    for h in range(2):
                        pt = fpsum.tile([P, P], dt.bfloat16, name="pt")
                        nc.tensor.transpose(
                            pt[:, :], vbf[i][:, h * P:(h + 1) * P], ident_sb[:, :])
                        nc.vector.tensor_copy(
                            vT[h][:, i * P:(i + 1) * P], pt[:, :])

                # FC: out[8b, 128o] in 4 concurrent col-groups, separate banks
                NCH = K * C // P  # 128 contraction chunks
                vTv = [vT[h].rearrange("p (b k) -> p k b", b=B) for h in range(2)]
                fcpg = [fcps.tile([P, OSL], dt.float32, name=f"fcp{g}", bufs=1)
                        for g in range(4)]
                for j in range(NCH):
                    grp = j % 4
                    lhsT = vTv[j % 2][:, j // 2, :]  # [128, 8] strided cols
                    nc.tensor.matmul(
                        fcpg[grp][32 * grp:32 * grp + B, :],
                        lhsT=lhsT,
                        rhs=fwt_sb[:, j * OSL:(j + 1) * OSL],
                        start=(j < 4), stop=(j >= NCH - 4),
                        tile_position=(0, 32 * grp),
                        skip_group_check=True,
                    )
                # gather the 4 partition-offset groups into one [128, OSL]
                # SBUF tile, then sum across partitions with a selector matmul
                sb4 = fpool.tile([P, OSL], dt.float32, name="sb4")
                nc.vector.memset(sb4[:, :], 0.0)
                for g in range(4):
                    nc.scalar.copy(
                        sb4[32 * g:32 * g + B, :],
                        fcpg[g][32 * g:32 * g + B, :])
                sel_sb = cpool.tile([P, B], dt.float32, name="sel_sb")
                nc.sync.dma_start(out=sel_sb[:, :], in_=sel_d[:, :])
                fcsum = fcps.tile([P, OSL], dt.float32, name="fcsum", bufs=1)
                nc.tensor.matmul(
                    fcsum[0:B, :], lhsT=sel_sb[:, :], rhs=sb4[:, :],
                    start=True, stop=True, skip_group_check=True,
                )
                fo = fpool.tile([B, OSL], dt.float32, name="fo")
                nc.vector.tensor_tensor(
                    out=fo[:, :], in0=fcsum[0:B, :], in1=fbb_sb[:, :],
                    op=ALU.add)

                # AllGather the [8, 128] slices
                # append per-core partial sum-of-squares as column OSL
                fop = fpool.tile([B, OSL + 1], dt.float32, name="fop")
                nc.vector.scalar_tensor_tensor(
                    out=fop[:, 0:OSL], in0=fo[:, :], scalar=1.0,
                    in1=fo[:, :], op0=ALU.mult, op1=ALU.mult,
                    accum_out=fop[:, OSL:OSL + 1])
                nc.vector.tensor_copy(fop[:, 0:OSL], fo[:, :])
                ag_in = dram.tile([B, OSL + 1], dt.float32, name="ag_in")
                ag_out = dram.tile([NCORES * B, OSL + 1], dt.float32, name="ag_out")
                nc.sync.dma_start(out=ag_in[:, :], in_=fop[:, :])
                nc.gpsimd.collective_compute(
                    "AllGather",
                    ALU.bypass,
                    replica_groups=[list(range(NCORES))],
                    ins=[ag_in[:, :]],
                    outs=[ag_out[:, :]],
                )
                # reassemble [8, 1024] + partial norms, final l2norm
                fin = fpool.tile([B, OUT], dt.float32, name="fin")
                agv = ag_out.rearrange("(c b) o -> b c o", b=B)
                nc.sync.dma_start(
                    out=fin.rearrange("b (c o) -> b c o", c=NCORES),
                    in_=agv[:, :, 0:OSL],
                )
                ssfp = fpool.tile([B, NCORES], dt.float32, name="ssfp")
                nc.sync.dma_start(out=ssfp[:, :], in_=agv[:, :, OSL])
                ssf = fpool.tile([B, 1], dt.float32, name="ssf")
                lnf = fpool.tile([B, 1], dt.float32, name="lnf")
                rnf = fpool.tile([B, 1], dt.float32, name="rnf")
                nc.vector.tensor_reduce(
                    out=ssf[:, :], in_=ssfp[:, :],
                    axis=mybir.AxisListType.X, op=ALU.add)
                nc.vector.tensor_scalar_max(ssf[:, :], ssf[:, :], 1e-24)
                nc.scalar.activation(lnf[:, :], ssf[:, :], AF.Ln)
                nc.scalar.activation(rnf[:, :], lnf[:, :], AF.Exp, scale=-0.5)
                fout = fpool.tile([B, OUT], dt.float32, name="fout")
                nc.vector.tensor_scalar(
                    out=fout[:, :], in0=fin[:, :],
                    scalar1=rnf[:, 0:1], scalar2=None, op0=ALU.mult)
                nc.sync.dma_start(out=out_d[:, :], in_=fout[:, :])

            vpool_ctx.__exit__(None, None, None)

    # Force every activation onto the one table set that holds Exp+Ln+Square
    # (+Copy/Identity) together -- the default per-function choice thrashes
    # ACT_TABLE_LOADs (~1.3us each) between exp_and_others / natural_log.
    import types
    import bass_rust as _bass_rust
    from concourse.hw_specs import get_activation_tables

    def _act_tables_one_set(self):
        has_activation = any(
            isinstance(i, mybir.InstActivation)
            for b in self.main_func.blocks
            for i in b.instructions
        )
        if not has_activation:
            return
        tables = get_activation_tables(self.m.arch)
        pref = "natural_log_exp_and_others"
        mod = [(k, (v if k == pref else set())) for k, v in tables.items()]
        _bass_rust.insert_act_table_loads(self, mod)

    nc.insert_act_table_loads = types.MethodType(_act_tables_one_set, nc)

    nc.compile()
    return nc


# ----------------------------------------------------------------------------
# Host-side input assembly per core
# ----------------------------------------------------------------------------

def _make_in_maps(feat, batch_ids, conv_w, conv_b, centroids, fc_w, fc_b):
    core_feat, Ts, n_pad = _plan(feat, batch_ids)
    corr = _pad_correction(conv_w, conv_b, n_pad)

    wt = np.ascontiguousarray(conv_w.T).astype(BF16)           # [256, 64]
    bb = np.broadcast_to(conv_b.astype(np.float32), (P, K)).copy()
    cent2 = np.concatenate([centroids, centroids], 0).astype(BF16)  # [128, 256]
    ident = np.eye(P, dtype=np.float32).astype(BF16)
    sel = np.zeros((P, B), np.float32)
    for g in range(4):
        for b in range(B):
            sel[32 * g + b, b] = 1.0

    OSL = OUT // NCORES
    in_maps = []
    for i in range(NCORES):
        cf = core_feat[i]
        nt = cf.shape[0] // P
        featN = np.ascontiguousarray(
            cf.reshape(nt, P, C).transpose(1, 0, 2)).astype(BF16)
        featT = np.ascontiguousarray(cf.T).astype(BF16)
        # fc slice, negated (vlad computed negated), chunk-major pre-swizzle:
        # fwt_sb[p, j*128+o] = -fc_w[o_base+o, j*128+p]
        fsl = -fc_w[i * OSL:(i + 1) * OSL]                      # [128, 16384]
        fsw = np.ascontiguousarray(
            fsl.reshape(OSL, K * C // P, P).transpose(2, 1, 0).reshape(P, K * C)
        ).astype(BF16)
        fbb = np.broadcast_to(fc_b[i * OSL:(i + 1) * OSL].astype(np.float32),
                              (B, OSL)).copy()
        in_maps.append({
            "featN": featN,
            "featT": featT,
            "wt": wt,
            "bb": bb,
            "cent2": cent2,
            "corr": corr,
            "fwt": fsw,
            "fbb": fbb,
            "ident": ident,
            "sel": sel,
        })
    return in_maps, Ts


def _ensure_profile_hook():
    """The agent image's `antenv` lacks `axon_hooks`; synthesize it so
    run_bass_kernel_spmd(trace=True) can reach the NTFF profiler."""
    import sys
    import types
    try:
        from antenv.axon_hooks import get_axon_ntff_profile_hook  # noqa: F401
        return True
    except ImportError:
        pass
    try:
        from trn_agent_boot.trn_boot import _ntff_profile_via_ctypes
        hook = _ntff_profile_via_ctypes("/opt/axon/libaxon_pjrt.so")
        if hook is None:
            return False
        mod = types.ModuleType("antenv.axon_hooks")
        mod._hook = hook
        mod.get_axon_ntff_profile_hook = lambda: mod._hook
        mod.set_axon_ntff_profile_hook = lambda h: setattr(mod, "_hook", h)
        import antenv
        antenv.axon_hooks = mod
        sys.modules["antenv.axon_hooks"] = mod
        return True
    except Exception:
        return False


def kernel(feat, batch_ids, centroids, conv_w, conv_b, fc_w, fc_b, batch_size):
    from concourse.bass_utils import run_bass_kernel_spmd

    feat = np.asarray(feat, dtype=np.float32)
    batch_ids = np.asarray(batch_ids, dtype=np.int32)
    centroids = np.asarray(centroids, dtype=np.float32)
    conv_w = np.asarray(conv_w, dtype=np.float32)
    conv_b = np.asarray(conv_b, dtype=np.float32)
    fc_w = np.asarray(fc_w, dtype=np.float32)
    fc_b = np.asarray(fc_b, dtype=np.float32)

    in_maps, Ts = _make_in_maps(
        feat, batch_ids, conv_w, conv_b, centroids, fc_w, fc_b)

    key = tuple(Ts)
    if key not in _compiled_cache:
        _compiled_cache[key] = _build_nc(Ts)
    nc = _compiled_cache[key]

    global LAST_RESULT
    do_trace = PROFILE and _ensure_profile_hook()
    res = run_bass_kernel_spmd(
        nc, in_maps, core_ids=list(range(NCORES)), trace=do_trace)
    LAST_RESULT = res
    return np.asarray(res.results[0]["out"], dtype=np.float32)



# revision 2
# speedup vs baseline: 1.2800x; 1.2800x over previous
"""NetVLAD Trainium2 kernel (8 NeuronCores, data-parallel over points).

Strategy (v2):
  - Host: l2-normalize feat -> x (removes the whole on-device 1/||x||
    pipeline), stable-sort points by batch_id, pad each batch to
    8*128*T_b rows with ZERO rows whose mask column is 0 (pads then
    contribute exactly nothing -> no correction step). Ship x twice in
    bf16: featN [P, TT, 257] (rows + mask col, aggregation operand) and
    featT [256, NP] (logits matmul operand).
  - conv_b spans ~[-9800, -7300] and needs ~0.05 abs precision, too
    much for bf16: split b = b_hi + b_mid + b_lo (3 bf16 rows) and fold
    it into the logits PSUM with one rank-3 matmul per group (free=512).
  - Device per group of G=8 tiles (1024 points), one PSUM bank [P,512]:
    bias matmul + 16 chunk matmuls -> biased logits; one batched negated
    row-max (DVE); 8x Exp with per-partition bias (ACT, psum->sbuf bf16);
    one batched Z reduce + reciprocal; 8x soft2 = e * (1/Z); 8x
    aggregation matmuls into per-batch-pair PSUM half-banks.
  - Per completed batch pair: evac psum -> bf16, AllReduce in bf16
    (values are O(1e3) partial sums; bf16 noise is ~0.4% of terms that
    only perturb vlad scale, not direction).
  - Tail: pairs' vlad normalize + transposes overlap the last pair's
    AllReduce; FC with per-core output slice (col-packed matmuls),
    AllGather, final l2norm.
"""

import numpy as np
import ml_dtypes

BF16 = ml_dtypes.bfloat16

N, C, K, B, OUT = 200000, 256, 64, 8, 1024
NCORES = 8
P = 128
G = 8  # tiles per group

_compiled_cache = {}
PROFILE = False       # set True to capture an NTFF profile (test harness only)
LAST_RESULT = None    # BassKernelResults of the most recent run


# ----------------------------------------------------------------------------
# Host-side planning
# ----------------------------------------------------------------------------

def _plan(feat, batch_ids):
    """Normalize rows, sort by batch, pad each batch to NCORES*P*T_b zero
    rows (mask col 0), build per-core shards."""
    nrm = np.sqrt(np.einsum("nc,nc->n", feat, feat, dtype=np.float64))
    x = feat / np.maximum(nrm, 1e-12)[:, None].astype(np.float32)

    order = np.argsort(batch_ids, kind="stable")
    x_s = x[order]
    counts = np.bincount(batch_ids, minlength=B)

    Ts = [int(np.ceil(c / (NCORES * P))) for c in counts]

    per_core = [[] for _ in range(NCORES)]
    per_core_mask = [[] for _ in range(NCORES)]
    off = 0
    for b in range(B):
        nb = int(counts[b])
        xb = x_s[off:off + nb]
        off += nb
        tot = NCORES * P * Ts[b]
        n_pad = tot - nb
        if n_pad:
            xb = np.concatenate([xb, np.zeros((n_pad, C), np.float32)], 0)
        mb = np.zeros((tot,), np.float32)
        mb[:nb] = 1.0
        xb = xb.reshape(NCORES, P * Ts[b], C)
        mb = mb.reshape(NCORES, P * Ts[b])
        for i in range(NCORES):
            per_core[i].append(xb[i])
            per_core_mask[i].append(mb[i])

    core_x = [np.concatenate(chunks, 0) for chunks in per_core]      # [NP, C]
    core_m = [np.concatenate(chunks, 0) for chunks in per_core_mask]  # [NP]
    return core_x, core_m, Ts


# ----------------------------------------------------------------------------
# Device program
# ----------------------------------------------------------------------------

def _build_nc(Ts):
    import concourse.bass as bass
    import concourse.bacc as bacc
    import concourse.mybir as mybir
    from concourse import tile

    dt = mybir.dt
    AF = mybir.ActivationFunctionType
    ALU = mybir.AluOpType

    TT = sum(Ts)            # tiles per core
    NP = TT * P             # points per core
    tile_batch = []         # batch id of each tile
    for b in range(B):
        tile_batch += [b] * Ts[b]

    nc = bacc.Bacc(
        "TRN2", target_bir_lowering=False, debug=False, num_devices=NCORES
    )

    # --- I/O ---
    featN_d = nc.dram_tensor("featN", [P, TT, C + 1], dt.bfloat16, kind="ExternalInput").ap()
    featT_d = nc.dram_tensor("featT", [C, NP], dt.bfloat16, kind="ExternalInput").ap()
    wt_d = nc.dram_tensor("wt", [C, K], dt.bfloat16, kind="ExternalInput").ap()
    b3_d = nc.dram_tensor("b3", [3, G * K], dt.bfloat16, kind="ExternalInput").ap()
    ones3_d = nc.dram_tensor("ones3", [3, P], dt.bfloat16, kind="ExternalInput").ap()
    cent2_d = nc.dram_tensor("cent2", [P, C], dt.bfloat16, kind="ExternalInput").ap()
    fwt_d = nc.dram_tensor("fwt", [P, K * C], dt.bfloat16, kind="ExternalInput").ap()
    fbb_d = nc.dram_tensor("fbb", [B, OUT // NCORES], dt.float32, kind="ExternalInput").ap()
    ident_d = nc.dram_tensor("ident", [P, P], dt.bfloat16, kind="ExternalInput").ap()
    sel_d = nc.dram_tensor("sel", [P, B], dt.float32, kind="ExternalInput").ap()
    out_d = nc.dram_tensor("out", [B, OUT], dt.float32, kind="ExternalOutput").ap()

    OSL = OUT // NCORES  # 128 output slice per core

    with tile.TileContext(nc) as tc:
        with (
            tc.tile_pool(name="const", bufs=1) as cpool,
            tc.tile_pool(name="dram", bufs=1, space="DRAM") as dram,
        ):
            wt_sb = cpool.tile([P, 2, K], dt.bfloat16, name="wt_sb")
            for h in range(2):
                nc.sync.dma_start(out=wt_sb[:, h, :],
                                  in_=wt_d[h * P:(h + 1) * P, :])
            b3_sb = cpool.tile([3, G * K], dt.bfloat16, name="b3_sb")
            nc.sync.dma_start(out=b3_sb[:, :], in_=b3_d[:, :])
            ones3_sb = cpool.tile([3, P], dt.bfloat16, name="ones3_sb")
            nc.sync.dma_start(out=ones3_sb[:, :], in_=ones3_d[:, :])
            cent2_sb = cpool.tile([P, C], dt.bfloat16, name="cent2_sb")
            nc.sync.dma_start(out=cent2_sb[:, :], in_=cent2_d[:, :])
            ident_sb = cpool.tile([P, P], dt.bfloat16, name="ident_sb")
            nc.sync.dma_start(out=ident_sb[:, :], in_=ident_d[:, :])
            fbb_sb = cpool.tile([B, OSL], dt.float32, name="fbb_sb")
            nc.sync.dma_start(out=fbb_sb[:, :], in_=fbb_d[:, :])
            fwt_sb = cpool.tile([P, K * C], dt.bfloat16, name="fwt_sb")
            for q in range(4):
                qs = K * C // 4
                nc.scalar.dma_start(out=fwt_sb[:, q * qs:(q + 1) * qs],
                                    in_=fwt_d[:, q * qs:(q + 1) * qs])

            # vlad-phase tiles that outlive the main-loop pools
            vpool_ctx = tc.tile_pool(name="vlad", bufs=1)
            vpool = vpool_ctx.__enter__()
            vbf = [vpool.tile([P, C], dt.bfloat16, name=f"vbf{i}")
                   for i in range(4)]
            ssv = vpool.tile([P, 4], dt.float32, name="ssv")
            lnv = vpool.tile([P, 4], dt.float32, name="lnv")
            rnv = vpool.tile([P, 4], dt.float32, name="rnv")

            # ---------------- main point loop ----------------
            with (
                tc.tile_pool(name="aggp", bufs=1, space="PSUM") as aggp,
                tc.tile_pool(name="psl", bufs=2, space="PSUM") as pslp,
                tc.tile_pool(name="grp", bufs=6) as gpool,
                tc.tile_pool(name="tl", bufs=8) as tpool,
            ):
                agg = [aggp.tile([P, C + 1], dt.float32, name=f"agg{i}")
                       for i in range(4)]

                t = 0
                last_tile_of_pair = [sum(Ts[:2 * pp + 2]) - 1 for pp in range(4)]
                evaced = [False] * 4

                part_d = dram.tile([B * K, C + 1], dt.bfloat16, name="part_d")
                red_d = dram.tile([B * K, C + 1], dt.bfloat16, name="red_d")

                while t < TT:
                    gs = min(G, TT - t)
                    featT_g = gpool.tile([P, 2, G * P], dt.bfloat16, name="featT_g")
                    featN_g = gpool.tile([P, G, C + 1], dt.bfloat16, name="featN_g")
                    negm_g = gpool.tile([P, G], dt.float32, name="negm_g")
                    z_g = gpool.tile([P, G], dt.float32, name="z_g")
                    rz_g = gpool.tile([P, G], dt.float32, name="rz_g")
                    e_g = gpool.tile([P, G * K], dt.bfloat16, name="e_g")
                    s2_g = gpool.tile([P, G * K], dt.bfloat16, name="s2_g")

                    # featT: two [128, gs*128] contiguous slabs (c-halves)
                    for h in range(2):
                        nc.sync.dma_start(
                            out=featT_g[:, h, 0:gs * P],
                            in_=featT_d[h * P:(h + 1) * P, t * P:(t + gs) * P],
                        )
                    # featN (with mask col): one DMA for the whole group
                    nc.sync.dma_start(
                        out=featN_g[:, 0:gs, :],
                        in_=featN_d[:, t:t + gs, :],
                    )

                    # biased logits for the whole group in one PSUM bank:
                    # bias via rank-3 matmul (b split into 3 bf16 rows)
                    psumL = pslp.tile([P, G * K], dt.float32, name="psumL")
                    nc.tensor.matmul(
                        psumL[:, 0:gs * K],
                        lhsT=ones3_sb[:, :],
                        rhs=b3_sb[:, 0:gs * K],
                        start=True, stop=False,
                        skip_group_check=True,
                    )
                    for g in range(gs):
                        for h in range(2):
                            nc.tensor.matmul(
                                psumL[:, (g * K):(g + 1) * K],
                                lhsT=featT_g[:, h, g * P:(g + 1) * P],
                                rhs=wt_sb[:, h, :],
                                start=False, stop=(h == 1),
                                skip_group_check=True,
                            )

                    # batched negated row max over the whole group
                    nc.vector.tensor_reduce(
                        out=negm_g[:, 0:gs],
                        in_=psumL.rearrange("p (g k) -> p g k", k=K)[:, 0:gs, :],
                        axis=mybir.AxisListType.X,
                        op=ALU.max,
                        negate=True,
                    )
                    # e = exp(t3 - m), psum -> sbuf bf16
                    for g in range(gs):
                        nc.scalar.activation(
                            e_g[:, g * K:(g + 1) * K],
                            psumL[:, g * K:(g + 1) * K],
                            AF.Exp,
                            bias=negm_g[:, g:g + 1],
                        )
                    # batched Z = sum_k e; rz = 1/Z
                    nc.vector.tensor_reduce(
                        out=z_g[:, 0:gs],
                        in_=e_g.rearrange("p (g k) -> p g k", k=K)[:, 0:gs, :],
                        axis=mybir.AxisListType.X,
                        op=ALU.add,
                    )
                    nc.vector.reciprocal(rz_g[:, 0:gs], z_g[:, 0:gs])

                    for g in range(gs):
                        tt = t + g
                        bb_idx = tile_batch[tt]
                        # soft2 = e * (1/Z)  (2 of 8 tiles on ACT to unload DVE)
                        if g % 4 == 3:
                            nc.scalar.mul(
                                s2_g[:, g * K:(g + 1) * K],
                                e_g[:, g * K:(g + 1) * K],
                                rz_g[:, g:g + 1])
                        else:
                            nc.vector.tensor_scalar(
                                out=s2_g[:, g * K:(g + 1) * K],
                                in0=e_g[:, g * K:(g + 1) * K],
                                scalar1=rz_g[:, g:g + 1],
                                scalar2=None,
                                op0=ALU.mult,
                            )
                        # aggregation matmul into per-batch half bank
                        pair, half = bb_idx // 2, bb_idx % 2
                        first = (tt == 0) or (tile_batch[tt - 1] != bb_idx)
                        last = (tt == TT - 1) or (tile_batch[tt + 1] != bb_idx)
                        nc.tensor.matmul(
                            agg[pair][half * K:(half + 1) * K, :],
                            lhsT=s2_g[:, g * K:(g + 1) * K],
                            rhs=featN_g[:, g, :],
                            start=first, stop=last,
                            tile_position=(0, half * K),
                        )
                    t += gs

                    # early evac + all-reduce per completed batch pair (bf16)
                    for pp in range(4):
                        if not evaced[pp] and last_tile_of_pair[pp] < t:
                            evaced[pp] = True
                            ev = tpool.tile([P, C + 1], dt.bfloat16, name="ev",
                                            tag="ev", bufs=2)
                            nc.scalar.copy(ev[:, :], agg[pp][:, :])
                            nc.sync.dma_start(
                                out=part_d[pp * P:(pp + 1) * P, :], in_=ev[:, :])
                            nc.gpsimd.collective_compute(
                                "AllReduce",
                                ALU.add,
                                replica_groups=[list(range(NCORES))],
                                ins=[part_d[pp * P:(pp + 1) * P, :]],
                                outs=[red_d[pp * P:(pp + 1) * P, :]],
                            )
                assert all(evaced)

            # ---------------- vlad + fc ----------------
            with (
                tc.tile_pool(name="fin", bufs=1) as fpool,
                tc.tile_pool(name="fps", bufs=2, space="PSUM") as fpsum,
                tc.tile_pool(name="fcp", bufs=1, space="PSUM") as fcps,
            ):
                vT = [fpool.tile([P, 4 * P], dt.bfloat16, name=f"vT{h}")
                      for h in range(2)]
                for i in range(4):
                    ared = fpool.tile([P, C + 1], dt.bfloat16, name="ared",
                                      tag="ared", bufs=2)
                    nc.sync.dma_start(out=ared[:, :],
                                      in_=red_d[i * P:(i + 1) * P, :])
                    aredf = fpool.tile([P, C + 1], dt.float32, name="aredf",
                                       tag="aredf", bufs=2)
                    nc.scalar.copy(aredf[:, :], ared[:, :])
                    # nv = cent*S - A   (negated vlad; fc weights negated)
                    nv = fpool.tile([P, C], dt.float32, name="nv", tag="nv",
                                    bufs=2)
                    nc.vector.scalar_tensor_tensor(
                        out=nv[:, :], in0=cent2_sb[:, :],
                        scalar=aredf[:, C:C + 1], in1=aredf[:, 0:C],
                        op0=ALU.mult, op1=ALU.subtract)
                    nvs = fpool.tile([P, C], dt.float32, name="nvs", tag="nvs",
                                     bufs=2)
                    nc.vector.scalar_tensor_tensor(
                        out=nvs[:, :], in0=nv[:, :], scalar=1.0, in1=nv[:, :],
                        op0=ALU.mult, op1=ALU.mult,
                        accum_out=ssv[:, i:i + 1])
                    nc.vector.tensor_scalar_max(
                        ssv[:, i:i + 1], ssv[:, i:i + 1], 1e-24)
                    nc.scalar.activation(lnv[:, i:i + 1], ssv[:, i:i + 1], AF.Ln)
                    nc.scalar.activation(rnv[:, i:i + 1], lnv[:, i:i + 1],
                                         AF.Exp, scale=-0.5)
                    nc.vector.tensor_scalar(
                        out=vbf[i][:, :], in0=nv[:, :],
                        scalar1=rnv[:, i:i + 1], scalar2=None, op0=ALU.mult)
                    # transpose the two c-halves into vT buffers
                    for h in range(2):
                        pt = fpsum.tile([P, P], dt.bfloat16, name="pt")
                        nc.tensor.transpose(
                            pt[:, :], vbf[i][:, h * P:(h + 1) * P], ident_sb[:, :])
                        nc.vector.tensor_copy(
                            vT[h][:, i * P:(i + 1) * P], pt[:, :])

                # FC: out[8b, 128o] in 4 concurrent col-groups, separate banks
                NCH = K * C // P  # 128 contraction chunks
                vTv = [vT[h].rearrange("p (b k) -> p k b", b=B) for h in range(2)]
                fcpg = [fcps.tile([P, OSL], dt.float32, name=f"fcp{g}", bufs=1)
                        for g in range(4)]
                for j in range(NCH):
                    grp = j % 4
                    lhsT = vTv[j % 2][:, j // 2, :]  # [128, 8] strided cols
                    nc.tensor.matmul(
                        fcpg[grp][32 * grp:32 * grp + B, :],
                        lhsT=lhsT,
                        rhs=fwt_sb[:, j * OSL:(j + 1) * OSL],
                        start=(j < 4), stop=(j >= NCH - 4),
                        tile_position=(0, 32 * grp),
                        skip_group_check=True,
                    )
                # gather the 4 partition-offset groups into one [128, OSL]
                # SBUF tile, then sum across partitions with a selector matmul
                sb4 = fpool.tile([P, OSL], dt.float32, name="sb4")
                nc.vector.memset(sb4[:, :], 0.0)
                for g in range(4):
                    nc.scalar.copy(
                        sb4[32 * g:32 * g + B, :],
                        fcpg[g][32 * g:32 * g + B, :])
                sel_sb = cpool.tile([P, B], dt.float32, name="sel_sb")
                nc.sync.dma_start(out=sel_sb[:, :], in_=sel_d[:, :])
                fcsum = fcps.tile([P, OSL], dt.float32, name="fcsum", bufs=1)
                nc.tensor.matmul(
                    fcsum[0:B, :], lhsT=sel_sb[:, :], rhs=sb4[:, :],
                    start=True, stop=True, skip_group_check=True,
                )
                fo = fpool.tile([B, OSL], dt.float32, name="fo")
                nc.vector.tensor_tensor(
                    out=fo[:, :], in0=fcsum[0:B, :], in1=fbb_sb[:, :],
                    op=ALU.add)

                # AllGather the [8, 128] slices
                # append per-core partial sum-of-squares as column OSL
                fop = fpool.tile([B, OSL + 1], dt.float32, name="fop")
                nc.vector.scalar_tensor_tensor(
                    out=fop[:, 0:OSL], in0=fo[:, :], scalar=1.0,
                    in1=fo[:, :], op0=ALU.mult, op1=ALU.mult,
                    accum_out=fop[:, OSL:OSL + 1])
                nc.vector.tensor_copy(fop[:, 0:OSL], fo[:, :])
                ag_in = dram.tile([B, OSL + 1], dt.float32, name="ag_in")
                ag_out = dram.tile([NCORES * B, OSL + 1], dt.float32, name="ag_out")
                nc.sync.dma_start(out=ag_in[:, :], in_=fop[:, :])
                nc.gpsimd.collective_compute(
                    "AllGather",
                    ALU.bypass,
                    replica_groups=[list(range(NCORES))],
                    ins=[ag_in[:, :]],
                    outs=[ag_out[:, :]],
                )
                # reassemble [8, 1024] + partial norms, final l2norm
                fin = fpool.tile([B, OUT], dt.float32, name="fin")
                agv = ag_out.rearrange("(c b) o -> b c o", b=B)
                nc.sync.dma_start(
                    out=fin.rearrange("b (c o) -> b c o", c=NCORES),
                    in_=agv[:, :, 0:OSL],
                )
                ssfp = fpool.tile([B, NCORES], dt.float32, name="ssfp")
                nc.sync.dma_start(out=ssfp[:, :], in_=agv[:, :, OSL])
                ssf = fpool.tile([B, 1], dt.float32, name="ssf")
                lnf = fpool.tile([B, 1], dt.float32, name="lnf")
                rnf = fpool.tile([B, 1], dt.float32, name="rnf")
                nc.vector.tensor_reduce(
                    out=ssf[:, :], in_=ssfp[:, :],
                    axis=mybir.AxisListType.X, op=ALU.add)
                nc.vector.tensor_scalar_max(ssf[:, :], ssf[:, :], 1e-24)
                nc.scalar.activation(lnf[:, :], ssf[:, :], AF.Ln)
                nc.scalar.activation(rnf[:, :], lnf[:, :], AF.Exp, scale=-0.5)
                fout = fpool.tile([B, OUT], dt.float32, name="fout")
                nc.vector.tensor_scalar(
                    out=fout[:, :], in0=fin[:, :],
                    scalar1=rnf[:, 0:1], scalar2=None, op0=ALU.mult)
                nc.sync.dma_start(out=out_d[:, :], in_=fout[:, :])

            vpool_ctx.__exit__(None, None, None)

    # Force every activation onto the one table set that holds Exp+Ln
    # (+Copy/Identity) together -- the default per-function choice thrashes
    # ACT_TABLE_LOADs (~1.3us each) between exp_and_others / natural_log.
    import types
    import bass_rust as _bass_rust
    from concourse.hw_specs import get_activation_tables

    def _act_tables_one_set(self):
        has_activation = any(
            isinstance(i, mybir.InstActivation)
            for b in self.main_func.blocks
            for i in b.instructions
        )
        if not has_activation:
            return
        tables = get_activation_tables(self.m.arch)
        pref = "natural_log_exp_and_others"
        mod = [(k, (v if k == pref else set())) for k, v in tables.items()]
        _bass_rust.insert_act_table_loads(self, mod)

    nc.insert_act_table_loads = types.MethodType(_act_tables_one_set, nc)

    nc.compile()
    return nc


# ----------------------------------------------------------------------------
# Host-side input assembly per core
# ----------------------------------------------------------------------------

def _make_in_maps(feat, batch_ids, conv_w, conv_b, centroids, fc_w, fc_b):
    core_x, core_m, Ts = _plan(feat, batch_ids)

    wt = np.ascontiguousarray(conv_w.T).astype(BF16)           # [256, 64]
    # conv_b split into 3 bf16 rows (exact to ~1e-4 abs), tiled G times
    b_hi = conv_b.astype(BF16)
    r1 = conv_b - b_hi.astype(np.float32)
    b_mid = r1.astype(BF16)
    r2 = r1 - b_mid.astype(np.float32)
    b_lo = r2.astype(BF16)
    b3 = np.stack([b_hi, b_mid, b_lo], 0)                       # [3, 64]
    b3t = np.tile(b3, (1, G))                                   # [3, 512]
    ones3 = np.ones((3, P), np.float32).astype(BF16)

    cent2 = np.concatenate([centroids, centroids], 0).astype(BF16)  # [128, 256]
    ident = np.eye(P, dtype=np.float32).astype(BF16)
    sel = np.zeros((P, B), np.float32)
    for g in range(4):
        for b in range(B):
            sel[32 * g + b, b] = 1.0

    OSL = OUT // NCORES
    in_maps = []
    for i in range(NCORES):
        cx = core_x[i]
        cm = core_m[i]
        nt = cx.shape[0] // P
        featN = np.empty((P, nt, C + 1), dtype=BF16)
        featN[:, :, 0:C] = cx.reshape(nt, P, C).transpose(1, 0, 2).astype(BF16)
        featN[:, :, C] = cm.reshape(nt, P).T.astype(BF16)
        featT = np.ascontiguousarray(cx.T).astype(BF16)
        # fc slice, negated (vlad computed negated), chunk-major pre-swizzle:
        # fwt_sb[p, j*128+o] = -fc_w[o_base+o, j*128+p]
        fsl = -fc_w[i * OSL:(i + 1) * OSL]                      # [128, 16384]
        fsw = np.ascontiguousarray(
            fsl.reshape(OSL, K * C // P, P).transpose(2, 1, 0).reshape(P, K * C)
        ).astype(BF16)
        fbb = np.broadcast_to(fc_b[i * OSL:(i + 1) * OSL].astype(np.float32),
                              (B, OSL)).copy()
        in_maps.append({
            "featN": featN,
            "featT": featT,
            "wt": wt,
            "b3": b3t,
            "ones3": ones3,
            "cent2": cent2,
            "fwt": fsw,
            "fbb": fbb,
            "ident": ident,
            "sel": sel,
        })
    return in_maps, Ts


def _ensure_profile_hook():
    """The agent image's `antenv` lacks `axon_hooks`; synthesize it so
    run_bass_kernel_spmd(trace=True) can reach the NTFF profiler."""
    import sys
    import types
    try:
        from antenv.axon_hooks import get_axon_ntff_profile_hook  # noqa: F401
        return True
    except ImportError:
        pass
    try:
        from trn_agent_boot.trn_boot import _ntff_profile_via_ctypes
        hook = _ntff_profile_via_ctypes("/opt/axon/libaxon_pjrt.so")
        if hook is None:
            return False
        mod = types.ModuleType("antenv.axon_hooks")
        mod._hook = hook
        mod.get_axon_ntff_profile_hook = lambda: mod._hook
        mod.set_axon_ntff_profile_hook = lambda h: setattr(mod, "_hook", h)
        import antenv
        antenv.axon_hooks = mod
        sys.modules["antenv.axon_hooks"] = mod
        return True
    except Exception:
        return False


def kernel(feat, batch_ids, centroids, conv_w, conv_b, fc_w, fc_b, batch_size):
    from concourse.bass_utils import run_bass_kernel_spmd

    feat = np.asarray(feat, dtype=np.float32)
    batch_ids = np.asarray(batch_ids, dtype=np.int32)
    centroids = np.asarray(centroids, dtype=np.float32)
    conv_w = np.asarray(conv_w, dtype=np.float32)
    conv_b = np.asarray(conv_b, dtype=np.float32)
    fc_w = np.asarray(fc_w, dtype=np.float32)
    fc_b = np.asarray(fc_b, dtype=np.float32)

    in_maps, Ts = _make_in_maps(
        feat, batch_ids, conv_w, conv_b, centroids, fc_w, fc_b)

    key = tuple(Ts)
    if key not in _compiled_cache:
        _compiled_cache[key] = _build_nc(Ts)
    nc = _compiled_cache[key]

    global LAST_RESULT
    do_trace = PROFILE and _ensure_profile_hook()
    res = run_bass_kernel_spmd(
        nc, in_maps, core_ids=list(range(NCORES)), trace=do_trace)
    LAST_RESULT = res
    return np.asarray(res.results[0]["out"], dtype=np.float32)
